# revision 11
# baseline (speedup 1.0000x reference)
"""Trainium2 Bass kernel for the CustomLSTM problem.

Contract: kernel(**inputs) takes the FULL unsharded numpy inputs
(x [4096,16,512] f32, per-gate weights/biases) and returns the FULL
output h_last [4096, 1024] f32.

Strategy (data-parallel over 8 NeuronCores):
  - shard batch B=4096 -> 512 per core; replicate weights.
  - per core, per timestep t, compute fused gates in transposed layout
    gT [4H=4096, B=512] as one PSUM accumulation per 128-row gate tile.
  - mixed precision over timesteps: early steps run both projections in
    fp8e4m3 with MatmulPerfMode.DoubleRow (2x128 contraction per
    instruction at 0.5 cycles/row -> ~4x bf16 matmul throughput); the
    last few steps run in bf16 because LSTM errors injected early decay
    through the forget gates while late-step errors hit the output
    directly.  Schedule: x-part bf16 for the last KX steps, h-part bf16
    for the last KHB steps.
  - resident SBUF weights: W8/U8 (fp8) + Wb (bf16).  The bf16 U does
    not fit on top of those, so it is streamed column-by-column from
    DRAM during the bf16 tail steps (gt-major host layout so each
    column is one contiguous-per-partition DMA).
  - gates run on ScalarE straight out of PSUM with the per-gate bias
    applied via the activation instruction's bias operand; sigmoid and
    tanh share one activation table set so the c-gate uses real Tanh.
  - the element-wise chain is 4 DVE ops per 128-row slice
    (t1=f*c, t2=i*ch, c=t1+t2, h=o*tanh(c)) in bf16 (2x DVE mode);
    c stays bf16 in SBUF; the final step writes h into an f32 tile that
    is DMA'd out.
"""

import numpy as np
import ml_dtypes

import concourse.bacc as bacc
import concourse.mybir as mybir
from concourse.tile import TileContext
from concourse.bass_utils import run_bass_kernel_spmd

F32 = mybir.dt.float32
BF16 = mybir.dt.bfloat16
F8 = mybir.dt.float8e4
AF = mybir.ActivationFunctionType
DR = mybir.MatmulPerfMode.DoubleRow

B, T, D, H = 4096, 16, 512, 1024
NCORES = 8
BL = B // NCORES          # batch per core
G = 4 * H                 # fused gate dim
KD = D // 128             # x contraction tiles
KH = H // 128             # h contraction tiles
NGT = G // 128            # gate tiles

KX = 4                    # last KX steps: x-part in bf16
KHB = 1                   # last KHB steps: h-part in bf16


def build_lstm(nc, reps=1, kx=KX, khb=KHB, bufs_g=4, bufs_x=2, bufs_h=3,
               bufs_tmp=3, bufs_us=8, t_steps=None, nodep=False):
    TS = t_steps if t_steps is not None else T
    x8_d = nc.declare_dram_parameter("x8", [T * D, BL], F8, isOutput=False)
    xb_d = nc.declare_dram_parameter("xb", [T * D, BL], BF16, isOutput=False)
    w8_d = nc.declare_dram_parameter("w8", [128, KD * G], F8, isOutput=False)
    u8_d = nc.declare_dram_parameter("u8", [128, KH * G], F8, isOutput=False)
    wb_d = nc.declare_dram_parameter("wb", [128, KD * G], BF16, isOutput=False)
    ubs_d = nc.declare_dram_parameter("ubs", [128, NGT * KH * 128], BF16,
                                      isOutput=False)
    b_d = nc.declare_dram_parameter("b", [128, NGT], F32, isOutput=False)
    out_d = nc.declare_dram_parameter("h_out", [H, BL], F32, isOutput=True)

    def xprec(t):
        return '8' if t < TS - kx else 'bf'

    def hprec(t):
        # precision of the h-projection performed AT step t (t >= 1)
        return '8' if t < TS - khb else 'bf'

    with TileContext(nc) as tc:
        with tc.tile_pool(name="const", bufs=1) as cpool, \
             tc.tile_pool(name="xp", bufs=bufs_x) as xpool, \
             tc.tile_pool(name="hp", bufs=bufs_h) as hpool, \
             tc.tile_pool(name="gp", bufs=bufs_g) as gpool, \
             tc.tile_pool(name="tp", bufs=bufs_tmp) as tpool, \
             tc.tile_pool(name="us", bufs=bufs_us) as uspool, \
             tc.tile_pool(name="ps", bufs=8, space="PSUM") as pspool:
            w8_sb = cpool.tile([128, KD, G], F8, name="w8_sb")
            nc.sync.dma_start(out=w8_sb[:], in_=w8_d[:])
            u8_sb = cpool.tile([128, KH, G], F8, name="u8_sb")
            wb_sb = cpool.tile([128, KD * G], BF16, name="wb_sb")

            def load_big():
                nc.sync.dma_start(out=u8_sb[:], in_=u8_d[:])
                nc.sync.dma_start(out=wb_sb[:], in_=wb_d[:])
            if reps != 1:
                load_big()
            b_sb = cpool.tile([128, NGT], F32, name="b_sb")
            nc.sync.dma_start(out=b_sb[:], in_=b_d[:])
            # c state, bf16; final h goes to the separate f32 out tile
            c_sb = cpool.tile([128, KH * BL], BF16, name="c_sb")
            o32_sb = cpool.tile([128, KH * BL], F32, name="o32_sb")

            MULT = mybir.AluOpType.mult
            SUB = mybir.AluOpType.subtract

            h_fake8 = h_fakeb = None
            if nodep:
                # timing-diagnostic mode: h matmuls read a constant tile
                # instead of the previous step's h (breaks the recurrence
                # dependency; numerics intentionally wrong)
                h_fake8 = cpool.tile([128, KH, BL], F8, name="h_fake8")
                nc.sync.dma_start(out=h_fake8[:],
                                  in_=u8_d[:, 0:KH * BL])
                h_fakeb = cpool.tile([128, KH, BL], BF16, name="h_fakeb")
                nc.sync.dma_start(out=h_fakeb[:], in_=ubs_d[:, 0:KH * BL])

            def body(rep):
                h_prev = None
                for t in range(TS):
                    xp = xprec(t)
                    x_t = xpool.tile([128, KD, BL], F8 if xp == '8' else BF16,
                                     name=f"x_{rep}_{t}", tag="x")
                    src = x8_d if xp == '8' else xb_d
                    for kd in range(KD):
                        nc.sync.dma_start(
                            out=x_t[:, kd, :],
                            in_=src[t * D + kd * 128: t * D + (kd + 1) * 128, :])
                    if t == 0 and reps == 1:
                        load_big()   # t=0 matmuls need only w8 + x_0
                    if t < TS - 1:
                        h_new = hpool.tile(
                            [128, KH, BL],
                            F8 if hprec(t + 1) == '8' else BF16,
                            name=f"h_{rep}_{t}", tag="h")
                    else:
                        h_new = None
                    hp = hprec(t)
                    pend = []
                    for ht in range(KH):
                        gates = gpool.tile([128, 4 * BL], BF16,
                                           name=f"gates_{rep}_{t}_{ht}",
                                           tag="g")
                        skip0 = 1 if t == 0 else 0  # f gate unused at t=0
                        pss = [(pspool.tile([128, BL], F32,
                                            name=f"ps_{rep}_{t}_{gi * KH + ht}",
                                            tag="ps")
                                if gi >= skip0 else None)
                               for gi in range(4)]

                        def mm_group(gis):
                            for gi in gis:
                                gt = gi * KH + ht
                                if xp == '8':
                                    for k2 in range(KD // 2):
                                        nc.tensor.matmul(
                                            pss[gi][:],
                                            w8_sb[:, 2 * k2:2 * k2 + 2,
                                                  gt * 128:(gt + 1) * 128],
                                            x_t[:, 2 * k2:2 * k2 + 2, :],
                                            start=(k2 == 0),
                                            stop=(t == 0 and
                                                  k2 == KD // 2 - 1),
                                            perf_mode=DR)
                                else:
                                    for kd in range(KD):
                                        nc.tensor.matmul(
                                            pss[gi][:],
                                            wb_sb[:, kd * G + gt * 128:
                                                  kd * G + gt * 128 + 128],
                                            x_t[:, kd, :],
                                            start=(kd == 0),
                                            stop=(t == 0 and kd == KD - 1))
                            if t > 0:
                                if hp == '8':
                                    # kh-major across gates: the freshest
                                    # h slices of the previous step are
                                    # consumed last.
                                    for k2 in range(KH // 2):
                                        for gi in gis:
                                            gt = gi * KH + ht
                                            nc.tensor.matmul(
                                                pss[gi][:],
                                                u8_sb[:, 2 * k2:2 * k2 + 2,
                                                      gt * 128:
                                                      (gt + 1) * 128],
                                                h_prev[:, 2 * k2:2 * k2 + 2,
                                                       :],
                                                start=False,
                                                stop=(k2 == KH // 2 - 1),
                                                perf_mode=DR)
                                else:
                                    uts = {}
                                    for gi in gis:
                                        gt = gi * KH + ht
                                        ut = uspool.tile(
                                            [128, KH, 128], BF16,
                                            name=f"us_{rep}_{t}_{gt}",
                                            tag="us")
                                        nc.sync.dma_start(
                                            out=ut[:],
                                            in_=ubs_d[:, gt * KH * 128:
                                                      (gt + 1) * KH * 128])
                                        uts[gi] = ut
                                    for kh in range(KH):
                                        for gi in gis:
                                            nc.tensor.matmul(
                                                pss[gi][:],
                                                uts[gi][:, kh, :],
                                                h_prev[:, kh, :],
                                                start=False,
                                                stop=(kh == KH - 1))

                        # i and the c-gate feed the DVE chain first: their
                        # matmul groups and activations go first, read
                        # directly from PSUM as truncated bf16 halfwords
                        # (the f32 PSUM port runs at half rate).  f and o
                        # drain through a DVE psum->sbuf copy + SBUF-side
                        # activation to split the PSUM port load across
                        # both engines.
                        mm_group([1, 3])
                        mm_group([0, 2] if t > 0 else [2])
                        for gi in (1, 3):
                            gt = gi * KH + ht
                            nc.scalar.activation(
                                gates[:, gi * BL:(gi + 1) * BL],
                                pss[gi][:].bitcast(BF16)[:, 1::2],
                                AF.Tanh if gi == 3 else AF.Sigmoid,
                                bias=b_sb[:, gt:gt + 1])
                        pz = tpool.tile([128, 2 * BL], BF16,
                                        name=f"pz_{rep}_{t}_{ht}", tag="pz",
                                        bufs=3)
                        for j, gi in enumerate((0, 2)):
                            if gi < skip0:
                                continue
                            gt = gi * KH + ht
                            nc.vector.tensor_copy(
                                pz[:, j * BL:(j + 1) * BL], pss[gi][:])
                            nc.scalar.activation(
                                gates[:, gi * BL:(gi + 1) * BL],
                                pz[:, j * BL:(j + 1) * BL],
                                AF.Sigmoid, bias=b_sb[:, gt:gt + 1])
                        gf = gates[:, 0 * BL:1 * BL]
                        gi_ = gates[:, 1 * BL:2 * BL]
                        go = gates[:, 2 * BL:3 * BL]
                        gc = gates[:, 3 * BL:4 * BL]
                        cs = c_sb[:, ht * BL:(ht + 1) * BL]
                        tmp = tpool.tile([128, 2 * BL], BF16,
                                         name=f"tmp_{rep}_{t}_{ht}", tag="tmp")
                        t1 = tmp[:, 0 * BL:1 * BL]
                        t2 = tmp[:, 1 * BL:2 * BL]
                        # c_new = f*c + i*ch
                        if t == 0:
                            nc.vector.tensor_mul(cs, gi_, gc)
                        else:
                            nc.vector.tensor_mul(t2, gi_, gc)
                            nc.vector.tensor_mul(t1, gf, cs)
                            nc.vector.tensor_add(cs, t1, t2)
                        # h = o*tanh(c), tanh computed once per ht pair
                        pend.append(go)
                        if ht % 2 == 1:
                            s2 = tpool.tile([128, 2 * BL], BF16,
                                            name=f"s2_{rep}_{t}_{ht}",
                                            tag="s2", bufs=2)
                            nc.scalar.activation(
                                s2[:], c_sb[:, (ht - 1) * BL:(ht + 1) * BL],
                                AF.Tanh)
                            for j, goj in enumerate(pend):
                                hj = ht - 1 + j
                                hdst = (h_new[:, hj, :] if t < TS - 1
                                        else o32_sb[:, hj * BL:(hj + 1) * BL])
                                nc.vector.tensor_mul(
                                    hdst, goj, s2[:, j * BL:(j + 1) * BL])
                            pend = []
                    if nodep:
                        h_prev = (h_fake8 if t + 1 < TS and
                                  hprec(t + 1) == '8' else h_fakeb)
                    else:
                        h_prev = h_new
                for kh in range(KH):
                    nc.sync.dma_start(out=out_d[kh * 128:(kh + 1) * 128, :],
                                      in_=o32_sb[:, kh * BL:(kh + 1) * BL])

            if reps == 1:
                body(0)
            else:
                with tc.For_i(0, reps, 1):
                    body(0)
    return nc


_BUILT = None


def _get_built():
    global _BUILT
    if _BUILT is None:
        nc = bacc.Bacc("TRN2", num_devices=NCORES)
        build_lstm(nc)
        nc.compile()
        _BUILT = nc
    return _BUILT


def _prep_inputs(x, wf, wi, wo, wc, uf, ui, uo, uc, bf, bi, bo, bc):
    bf16 = ml_dtypes.bfloat16
    e4 = ml_dtypes.float8_e4m3
    W = np.concatenate([wf, wi, wo, wc], axis=1)                   # [D, 4H]
    U = np.concatenate([uf, ui, uo, uc], axis=1)                   # [H, 4H]
    b = np.concatenate([bf, bi, bo, bc], axis=1).astype(np.float32)
    b_t = np.ascontiguousarray(b.reshape(NGT, 128).T)              # [128, NGT]
    # weight images: (p, k, g) = W[k*128+p, g], flattened per partition
    w8 = np.ascontiguousarray(
        W.reshape(KD, 128, G).transpose(1, 0, 2).reshape(128, KD * G)
    ).astype(e4)
    u8 = np.ascontiguousarray(
        U.reshape(KH, 128, G).transpose(1, 0, 2).reshape(128, KH * G)
    ).astype(e4)
    wb = np.ascontiguousarray(
        W.reshape(KD, 128, G).transpose(1, 0, 2).reshape(128, KD * G)
    ).astype(bf16)
    # streamed bf16 U, gt-major: (p, gt, kh, m) = U[kh*128+p, gt*128+m]
    ubs = np.ascontiguousarray(
        U.reshape(KH, 128, NGT, 128).transpose(1, 2, 0, 3).reshape(128, -1)
    ).astype(bf16)
    # x [B, T, D] -> per-core [T*D, BL] with xT[t*D+d, b] = x[b, t, d]
    xt = np.ascontiguousarray(np.transpose(x, (1, 2, 0)))  # [T, D, B] f32
    in_maps = []
    for c in range(NCORES):
        xc = np.ascontiguousarray(
            xt[:, :, c * BL:(c + 1) * BL].reshape(T * D, BL))
        in_maps.append({"x8": xc.astype(e4), "xb": xc.astype(bf16),
                        "w8": w8, "u8": u8, "wb": wb, "ubs": ubs,
                        "b": b_t})
    return in_maps


def kernel(x, wf, wi, wo, wc, uf, ui, uo, uc, bf, bi, bo, bc):
    nc = _get_built()
    in_maps = _prep_inputs(x, wf, wi, wo, wc, uf, ui, uo, uc, bf, bi, bo, bc)
    res = run_bass_kernel_spmd(nc, in_maps, list(range(NCORES)))
    out = np.empty((B, H), np.float32)
    for c in range(NCORES):
        out[c * BL:(c + 1) * BL, :] = res.results[c]["h_out"].T
    return out
